# revision 30
# baseline (speedup 1.0000x reference)
"""Trainium2 Bass kernel for the ConvShapeletModel problem.

Pipeline (per core, data-parallel over batch: 32 -> 8 cores x 4):
  conv1d(x, w_s) for 4 kernel widths  -> cummax over time -> batchnorm
  -> logreg head (10 cls) + dec head (1) -> softmax(cls), softmax over T,
  cumprod budget recurrence -> (probs, pts)

Key tricks:
  * BN is folded into conv weights on the host; conv bias + BN shift are
    folded into the head bias (cummax commutes with per-channel affine
    monotone maps, biases are constant over t).
  * conv = matmul over an im2col built from 9 column-shifted replicas of
    x in SBUF: every (scale, tap-group) is a single AP slice.
  * fp32r matmuls: full fp32 data, 1 cycle/row at N=512.
  * cummax and the cumprod budget recurrence use the hardware
    tensor_tensor_scan instruction (one pass per row).
"""

import os

os.environ.setdefault("MYCRO_LOCAL_CACHE", "1")

import numpy as np

import concourse.bass as bass
import concourse.tile as tile
from concourse import bacc, mybir
from concourse.bass_utils import run_bass_kernel_spmd
from concourse.masks import make_identity

F32 = mybir.dt.float32
F32R = mybir.dt.float32r
AX = mybir.AxisListType
ALU = mybir.AluOpType
ACTF = mybir.ActivationFunctionType

# Problem shapes (hardcoded).
B_FULL, C, T, H, NCLS = 32, 13, 2048, 128, 10
N_CORES = 8
B = B_FULL // N_CORES  # 4 batches per core
KS = [10, 20, 30, 40]
NSCALE = 4
NREP = 9  # column-shifted replicas of x (9*13 = 117 <= 128 partitions)
PAD = 20  # max K//2
XREP_W = T + 2 * PAD  # 2088
XPAD_W = XREP_W + NREP - 1  # 2096, host-padded x width (zero halo included)
NT = T // 128  # 16 t-tiles of 128
NHEAD = NCLS + 1  # 10 logreg cols + 1 dec col

# Tap groups per scale: (global_group_idx, n_rows, k0, group_size)
GROUPS = []
_gidx = 0
for _K in KS:
    gl = []
    for _k0 in range(0, _K, NREP):
        _gs = min(NREP, _K - _k0)
        gl.append((_gidx, 13 * _gs, _k0, _gs))
        _gidx += 1
    GROUPS.append(gl)
NGROUPS = _gidx  # 14

LAST_EXEC_NS = None
_CACHED_NC = None


def _emit(tc, ctx):
    nc = tc.nc
    # x / wconv feed fp32r matmuls; declare them fp32r end-to-end (same bits
    # as fp32 host-side — dt.np(float32r) is np.float32) so the BIR verifier
    # sees a consistent fp32r producer chain.
    x_d = nc.dram_tensor("x", [B, C, XPAD_W], F32R, kind="ExternalInput").ap()
    wconv_d = nc.dram_tensor(
        "wconv", [13 * NREP, NGROUPS * H], F32R, kind="ExternalInput"
    ).ap()
    whead_d = nc.dram_tensor(
        "whead", [H, NSCALE * NHEAD], F32, kind="ExternalInput"
    ).ap()
    biasbc_d = nc.dram_tensor(
        "biasbc", [128, NT * NHEAD], F32, kind="ExternalInput"
    ).ap()
    probs_d = nc.dram_tensor("probs", [B, T, NCLS], F32, kind="ExternalOutput").ap()
    pts_d = nc.dram_tensor("pts", [B, T], F32, kind="ExternalOutput").ap()

    singles = ctx.enter_context(tc.tile_pool(name="singles", bufs=1))
    xrep_pool = ctx.enter_context(tc.tile_pool(name="xrep", bufs=2))
    feat_pool = ctx.enter_context(tc.tile_pool(name="feat", bufs=2))
    work_pool = ctx.enter_context(tc.tile_pool(name="work", bufs=2))
    convps_pool = ctx.enter_context(tc.tile_pool(name="convps", bufs=2, space="PSUM"))
    headps_pool = ctx.enter_context(tc.tile_pool(name="headps", bufs=2, space="PSUM"))
    trps_pool = ctx.enter_context(tc.tile_pool(name="trps", bufs=2, space="PSUM"))

    # ---- constants ----
    identity = singles.tile([128, 128], F32)
    make_identity(nc, identity)

    zeros = singles.tile([128, T], F32)
    nc.gpsimd.memset(zeros, 0.0)

    # all 14 tap-group weight blocks packed side by side in the free dim
    wc_all = singles.tile([13 * NREP, NGROUPS * H], F32R)
    nc.sync.dma_start(out=wc_all, in_=wconv_d)

    whead_all = singles.tile([H, NSCALE * NHEAD], F32)
    nc.sync.dma_start(out=whead_all, in_=whead_d)

    bias_bc = singles.tile([128, NT * NHEAD], F32)
    nc.sync.dma_start(out=bias_bc, in_=biasbc_d)

    # dec-head logits, one row per batch, time along free dim
    D = singles.tile([B, T], F32)
    # dec-column staging: col (b*NT + j) holds batch b, t-tile j
    dcols = singles.tile([128, B * NT], F32)

    # ---- per-batch pipeline ----
    for b in range(B):
        xrep = xrep_pool.tile([13 * NREP, XREP_W], F32R, tag="xrep")
        xbase = x_d[b, :, 0:XREP_W]
        xin = bass.AP(
            tensor=xbase.tensor,
            offset=xbase.offset,
            ap=[[1, NREP]] + list(xbase.ap),
        )
        nc.sync.dma_start(out=xrep, in_=xin)

        feats = []
        for s in range(NSCALE):
            K = KS[s]
            feat = feat_pool.tile([H, T], F32, tag=f"feat{s}", name=f"feat_{b}_{s}")
            for half in range(2):
                ps = convps_pool.tile([128, 1024], F32, tag="convps")
                for tcol in range(2):
                    t0 = half * 1024 + tcol * 512
                    glist = GROUPS[s]
                    for gi, (gg, nrows, k0, gs) in enumerate(glist):
                        n0 = t0 + PAD + k0 - K // 2
                        nc.tensor.matmul(
                            ps[:, tcol * 512 : (tcol + 1) * 512],
                            lhsT=wc_all[0:nrows, gg * H : (gg + 1) * H],
                            rhs=xrep[0:nrows, n0 : n0 + 512],
                            start=(gi == 0),
                            stop=(gi == len(glist) - 1),
                        )
                init = -1e30 if half == 0 else feat[:, 1023:1024]
                nc.vector.tensor_tensor_scan(
                    out=feat[:, half * 1024 : (half + 1) * 1024],
                    data0=ps[:, 0:1024],
                    data1=zeros[:, 0:1024],
                    initial=init,
                    op0=ALU.max,
                    op1=ALU.bypass,
                )
            feats.append(feat)

        # head: logits[t, c] for 16 t-tiles x 11 cols, one PSUM bank
        ps_head = headps_pool.tile([128, NT * NHEAD], F32, tag="headps")
        for j in range(NT):
            for s in range(NSCALE):
                nc.tensor.matmul(
                    ps_head[:, j * NHEAD : (j + 1) * NHEAD],
                    lhsT=feats[s][:, j * 128 : (j + 1) * 128],
                    rhs=whead_all[:, s * NHEAD : (s + 1) * NHEAD],
                    start=(s == 0),
                    stop=(s == NSCALE - 1),
                )

        logits = work_pool.tile([128, NT * NHEAD], F32, tag="logits")
        nc.vector.tensor_tensor(out=logits, in0=ps_head, in1=bias_bc, op=ALU.add)

        ex = work_pool.tile([128, NT * NHEAD], F32, tag="ex")
        nc.scalar.activation(ex, logits, ACTF.Exp)

        ex3 = ex.rearrange("p (j c) -> p j c", c=NHEAD)
        ssum = work_pool.tile([128, NT], F32, tag="ssum")
        nc.vector.tensor_reduce(
            out=ssum, in_=ex3[:, :, 0:NCLS], axis=AX.X, op=ALU.add
        )
        rs = work_pool.tile([128, NT], F32, tag="rs")
        nc.vector.reciprocal(out=rs, in_=ssum)

        probs_sb = work_pool.tile([128, NT * NCLS], F32, tag="probs_sb")
        for j in range(NT):
            nc.vector.tensor_scalar_mul(
                probs_sb[:, j * NCLS : (j + 1) * NCLS],
                ex[:, j * NHEAD : j * NHEAD + NCLS],
                rs[:, j : j + 1],
            )
        nc.scalar.dma_start(
            out=probs_d[b].rearrange("(j p) c -> p j c", p=128),
            in_=probs_sb.rearrange("p (j c) -> p j c", c=NCLS),
        )

        # stage this batch's dec column (strided col 10 of each 11-group)
        lg3 = logits.rearrange("p (j c) -> p j c", c=NHEAD)
        nc.vector.tensor_copy(
            out=dcols[:, b * NT : (b + 1) * NT].rearrange("p (j o) -> p j o", o=1),
            in_=lg3[:, :, NCLS : NCLS + 1],
        )

    # one transpose for all batches: [128, 64] -> [64, 128]
    tr = trps_pool.tile([B * NT, 128], F32, tag="trps")
    nc.tensor.transpose(out=tr, in_=dcols, identity=identity)
    dtmp_all = singles.tile([B * NT, 128], F32)
    nc.vector.tensor_copy(out=dtmp_all, in_=tr)
    nc.scalar.dma_start(
        out=D.rearrange("b (j p) -> b j p", p=128),
        in_=dtmp_all,
    )

    # ---- deltas tail: softmax over T, budget cumprod, pts ----
    m = singles.tile([B, 1], F32)
    nc.vector.tensor_reduce(out=m, in_=D, axis=AX.X, op=ALU.max)
    negm = singles.tile([B, 1], F32)
    nc.scalar.activation(negm, m, ACTF.Copy, bias=0.0, scale=-1.0)

    ed = singles.tile([B, T], F32)
    ssumd = singles.tile([B, 1], F32)
    nc.scalar.activation(ed, D, ACTF.Exp, bias=negm, scale=1.0, accum_out=ssumd)
    rd = singles.tile([B, 1], F32)
    nc.vector.reciprocal(out=rd, in_=ssumd)
    delta = singles.tile([B, T], F32)
    nc.vector.tensor_scalar_mul(delta, ed, rd)

    q = singles.tile([B, T], F32)
    nc.scalar.activation(q, delta, ACTF.Copy, bias=1.0, scale=-1.0)

    sc = singles.tile([B, T - 1], F32)
    nc.vector.tensor_tensor_scan(
        out=sc,
        data0=q[:, 1:T],
        data1=zeros[0:B, 0 : T - 1],
        initial=1.0,
        op0=ALU.mult,
        op1=ALU.bypass,
    )

    pts_sb = singles.tile([B, T], F32)
    nc.vector.tensor_copy(out=pts_sb[:, 0:1], in_=delta[:, 1:2])
    nc.vector.tensor_mul(pts_sb[:, 1 : T - 1], delta[:, 2:T], sc[:, 0 : T - 2])
    nc.vector.tensor_copy(out=pts_sb[:, T - 1 : T], in_=sc[:, T - 2 : T - 1])
    nc.scalar.dma_start(out=pts_d, in_=pts_sb)


def build_nc():
    global _CACHED_NC
    if _CACHED_NC is not None:
        return _CACHED_NC
    nc = bacc.Bacc(
        "TRN2", target_bir_lowering=False, debug=False, num_devices=N_CORES
    )
    from contextlib import ExitStack

    with tile.TileContext(nc) as tc, ExitStack() as ctx:
        _emit(tc, ctx)
    nc.compile()
    _CACHED_NC = nc
    return nc


def host_prep(inputs):
    """Fold BN + biases; pack conv/head weights. Returns per-core param dict."""
    f32 = np.float32
    gamma = np.asarray(inputs["bn_gamma"], f32)
    beta = np.asarray(inputs["bn_beta"], f32)
    mean = np.asarray(inputs["bn_mean"], f32)
    var = np.asarray(inputs["bn_var"], f32)
    a = (gamma / np.sqrt(var + np.float32(1e-5))).astype(f32)  # [512]
    cshift = (beta - mean * a).astype(f32)

    ws = [np.asarray(inputs[f"w{i}"], f32) for i in range(1, 5)]
    bs = [np.asarray(inputs[f"b{i}"], f32) for i in range(1, 5)]

    cb = np.zeros(4 * H, f32)  # per-feature constant shift (conv bias + BN)
    wconv = np.zeros((13 * NREP, NGROUPS * H), f32)
    for s, (w, bias, K) in enumerate(zip(ws, bs, KS)):
        asl = a[s * H : (s + 1) * H]
        csl = cshift[s * H : (s + 1) * H]
        wp = (w * asl[:, None, None]).astype(f32)  # [H, C, K]
        cb[s * H : (s + 1) * H] = bias * asl + csl
        wt = np.ascontiguousarray(np.transpose(wp, (2, 1, 0)))  # [K, C, H]
        for gg, nrows, k0, gs in GROUPS[s]:
            wconv[:nrows, gg * H : (gg + 1) * H] = wt[k0 : k0 + gs].reshape(
                gs * C, H
            )

    logreg_w = np.asarray(inputs["logreg_w"], f32)  # [10, 512]
    logreg_b = np.asarray(inputs["logreg_b"], f32)
    dec_w = np.asarray(inputs["dec_w"], f32)  # [1, 512]

    # [128, 4*11]: chunk s at cols [s*11, (s+1)*11)
    whead = np.zeros((H, NSCALE * NHEAD), f32)
    for s in range(NSCALE):
        whead[:, s * NHEAD : s * NHEAD + NCLS] = logreg_w.T[s * H : (s + 1) * H]
        whead[:, s * NHEAD + NCLS] = dec_w[0, s * H : (s + 1) * H]

    hb = np.zeros(NHEAD, f32)
    hb[:NCLS] = logreg_b + logreg_w @ cb  # dec-col bias is softmax-invariant
    biasbc = np.broadcast_to(np.tile(hb, NT), (128, NT * NHEAD))
    biasbc = np.ascontiguousarray(biasbc, dtype=f32)

    return {"wconv": wconv, "whead": whead, "biasbc": biasbc}


def make_in_maps(inputs):
    params = host_prep(inputs)
    x = np.asarray(inputs["x"], np.float32)
    xpad = np.zeros((B_FULL, C, XPAD_W), np.float32)
    xpad[:, :, PAD : PAD + T] = x
    in_maps = []
    for i in range(N_CORES):
        m = {"x": np.ascontiguousarray(xpad[i * B : (i + 1) * B])}
        m.update(params)
        in_maps.append(m)
    return in_maps


def kernel(**inputs):
    global LAST_EXEC_NS
    nc = build_nc()
    in_maps = make_in_maps(inputs)
    res = run_bass_kernel_spmd(nc, in_maps, list(range(N_CORES)))
    LAST_EXEC_NS = res.exec_time_ns
    probs = np.concatenate([res.results[i]["probs"] for i in range(N_CORES)], 0)
    pts = np.concatenate([res.results[i]["pts"] for i in range(N_CORES)], 0)
    return probs, pts


def bench(inputs, iters=64, warmup=8):
    """Amortized per-call wall time of the compiled NEFF across 8 cores.

    No NTFF profiling is available through the axon tunnel in this
    container, so this times back-to-back PJRT executions with inputs
    resident on device (no donation, outputs written fully by the
    kernel) and reports the steady-state per-call time.
    """
    import time

    import jax
    from jax.sharding import Mesh, PartitionSpec
    from jax.experimental.shard_map import shard_map

    from concourse import bass2jax
    from concourse import mybir as mb

    nc = build_nc()
    in_maps = make_in_maps(inputs)
    bass2jax.install_neuronx_cc_hook()

    partition_name = (
        nc.partition_id_tensor.name if nc.partition_id_tensor else None
    )
    in_names, out_names, out_avals, zero_outs = [], [], [], []
    for alloc in nc.m.functions[0].allocations:
        if not isinstance(alloc, mb.MemoryLocationSet):
            continue
        name = alloc.memorylocations[0].name
        if alloc.kind == "ExternalInput":
            if name != partition_name:
                in_names.append(name)
        elif alloc.kind == "ExternalOutput":
            out_names.append(name)
            out_avals.append(
                jax.core.ShapedArray(alloc.tensor_shape, mb.dt.np(alloc.dtype))
            )
            zero_outs.append(np.zeros(alloc.tensor_shape, mb.dt.np(alloc.dtype)))
    n_params = len(in_names)
    all_names = in_names + out_names
    if partition_name is not None:
        all_names = all_names + [partition_name]

    def _body(*args):
        operands = list(args)
        if partition_name is not None:
            operands.append(bass2jax.partition_id_tensor())
        outs = bass2jax._bass_exec_p.bind(
            *operands,
            out_avals=tuple(out_avals),
            in_names=tuple(all_names),
            out_names=tuple(out_names),
            lowering_input_output_aliases=(),
            sim_require_finite=True,
            sim_require_nnan=True,
            nc=nc,
        )
        return tuple(outs)

    devices = jax.devices()[:N_CORES]
    mesh = Mesh(np.asarray(devices), ("core",))
    nin = n_params + len(out_names)
    sharded = jax.jit(
        shard_map(
            _body,
            mesh=mesh,
            in_specs=(PartitionSpec("core"),) * nin,
            out_specs=(PartitionSpec("core"),) * len(out_names),
            check_rep=False,
        ),
        keep_unused=True,
    )
    # shard along axis0: per-core shard must equal the declared per-core shape
    concat_in = [
        np.concatenate([np.asarray(in_maps[c][n]) for c in range(N_CORES)], 0)
        for n in in_names
    ]
    concat_zeros = [
        np.zeros((N_CORES * z.shape[0], *z.shape[1:]), z.dtype) for z in zero_outs
    ]
    from jax.sharding import NamedSharding

    sh = NamedSharding(mesh, PartitionSpec("core"))
    dev_in = [jax.device_put(a, sh) for a in concat_in + concat_zeros]

    for _ in range(warmup):
        out = sharded(*dev_in)
    jax.block_until_ready(out)
    t0 = time.perf_counter()
    for _ in range(iters):
        out = sharded(*dev_in)
    jax.block_until_ready(out)
    t1 = time.perf_counter()
    return (t1 - t0) / iters * 1e9


# revision 49
# speedup vs baseline: 1.0574x; 1.0574x over previous
"""Trainium2 Bass kernel for the ConvShapeletModel problem.

Pipeline (per core, data-parallel over batch: 32 -> 8 cores x 4):
  conv1d(x, w_s) for 4 kernel widths  -> cummax over time -> batchnorm
  -> logreg head (10 cls) + dec head (1) -> softmax(cls), softmax over T,
  cumprod budget recurrence -> (probs, pts)

Key tricks:
  * BN is folded into conv weights on the host; conv bias + BN shift are
    folded into the head bias (cummax commutes with per-channel affine
    monotone maps, biases are constant over t).
  * conv = matmul over an im2col built from 9 column-shifted replicas of
    x in SBUF: every (scale, tap-group) is a single AP slice.
  * fp32r matmuls: full fp32 data, 1 cycle/row at N=512.
  * cummax and the cumprod budget recurrence use the hardware
    tensor_tensor_scan instruction (one pass per row).
"""

import os

os.environ.setdefault("MYCRO_LOCAL_CACHE", "1")

import numpy as np

import concourse.bass as bass
import concourse.tile as tile
from concourse import bacc, mybir
from concourse.bass_utils import run_bass_kernel_spmd
from concourse.masks import make_identity

F32 = mybir.dt.float32
F32R = mybir.dt.float32r
BF16 = mybir.dt.bfloat16
AX = mybir.AxisListType
ALU = mybir.AluOpType
ACTF = mybir.ActivationFunctionType

# Problem shapes (hardcoded).
B_FULL, C, T, H, NCLS = 32, 13, 2048, 128, 10
N_CORES = 8
B = B_FULL // N_CORES  # 4 batches per core
KS = [10, 20, 30, 40]
NSCALE = 4
NREP = 9  # column-shifted replicas of x (9*13 = 117 <= 128 partitions)
PAD = 20  # max K//2
XREP_W = T + 2 * PAD  # 2088
XPAD_W = XREP_W + NREP - 1  # 2096, host-padded x width (zero halo included)
NT = T // 128  # 16 t-tiles of 128
NHEAD = NCLS + 1  # 10 logreg cols + 1 dec col

# Tap groups per scale: (global_group_idx, n_rows, k0, group_size)
GROUPS = []
_gidx = 0
for _K in KS:
    gl = []
    for _k0 in range(0, _K, NREP):
        _gs = min(NREP, _K - _k0)
        gl.append((_gidx, 13 * _gs, _k0, _gs))
        _gidx += 1
    GROUPS.append(gl)
NGROUPS = _gidx  # 14

LAST_EXEC_NS = None
_CACHED_NC = None


def _emit(tc, ctx, niter=1):
    nc = tc.nc
    # x / wconv feed fp32r matmuls; declare them fp32r end-to-end (same bits
    # as fp32 host-side — dt.np(float32r) is np.float32) so the BIR verifier
    # sees a consistent fp32r producer chain.
    x_d = nc.dram_tensor("x", [B, C, XPAD_W], F32R, kind="ExternalInput").ap()
    wconv_d = nc.dram_tensor(
        "wconv", [13 * NREP, NGROUPS * H], F32R, kind="ExternalInput"
    ).ap()
    whead_d = nc.dram_tensor(
        "whead", [H, NSCALE * NHEAD], BF16, kind="ExternalInput"
    ).ap()
    biasbc_d = nc.dram_tensor(
        "biasbc", [128, NT * NHEAD], F32, kind="ExternalInput"
    ).ap()
    probs_d = nc.dram_tensor("probs", [B, T, NCLS], F32, kind="ExternalOutput").ap()
    pts_d = nc.dram_tensor("pts", [B, T], F32, kind="ExternalOutput").ap()

    singles = ctx.enter_context(tc.tile_pool(name="singles", bufs=1))
    xrep_pool = ctx.enter_context(tc.tile_pool(name="xrep", bufs=2))
    feat_pool = ctx.enter_context(tc.tile_pool(name="feat", bufs=2))
    work_pool = ctx.enter_context(tc.tile_pool(name="work", bufs=2))
    convps_pool = ctx.enter_context(tc.tile_pool(name="convps", bufs=3, space="PSUM"))
    headps_pool = ctx.enter_context(tc.tile_pool(name="headps", bufs=1, space="PSUM"))
    trps_pool = ctx.enter_context(tc.tile_pool(name="trps", bufs=1, space="PSUM"))

    # ---- constants ----
    identity = singles.tile([128, 128], F32)
    make_identity(nc, identity)

    zeros = singles.tile([128, T], F32)
    nc.gpsimd.memset(zeros, 0.0)

    # all 14 tap-group weight blocks packed side by side in the free dim
    wc_all = singles.tile([13 * NREP, NGROUPS * H], F32R)
    nc.sync.dma_start(out=wc_all, in_=wconv_d)

    whead_all = singles.tile([H, NSCALE * NHEAD], BF16)
    nc.scalar.dma_start(out=whead_all, in_=whead_d)

    bias_bc = singles.tile([128, NT * NHEAD], F32)
    nc.scalar.dma_start(out=bias_bc, in_=biasbc_d)

    # dec-column staging: col (b*NT + j) holds batch b, t-tile j
    dcols = singles.tile([128, B * NT], F32)

    for _it in range(niter):
        _emit_body(tc, ctx, locals())


def _emit_body(tc, ctx, env):
    nc = tc.nc
    singles = env["singles"]
    xrep_pool = env["xrep_pool"]
    feat_pool = env["feat_pool"]
    work_pool = env["work_pool"]
    convps_pool = env["convps_pool"]
    headps_pool = env["headps_pool"]
    trps_pool = env["trps_pool"]
    identity = env["identity"]
    zeros = env["zeros"]
    wc_all = env["wc_all"]
    whead_all = env["whead_all"]
    bias_bc = env["bias_bc"]
    dcols = env["dcols"]
    x_d = env["x_d"]
    probs_d = env["probs_d"]
    pts_d = env["pts_d"]

    # ---- per-batch pipeline ----
    for b in range(B):
        xrep = xrep_pool.tile([13 * NREP, XREP_W], F32R, tag="xrep")
        # batch 0's replica build is on the critical path: split it into
        # column chunks so the first conv matmuls start sooner
        col_chunks = [(0, 548), (548, 1060), (1060, XREP_W)] if b == 0 else [
            (0, XREP_W)
        ]
        for c0, c1 in col_chunks:
            xbase = x_d[b, :, c0:c1]
            xin = bass.AP(
                tensor=xbase.tensor,
                offset=xbase.offset,
                ap=[[1, NREP]] + list(xbase.ap),
            )
            nc.sync.dma_start(out=xrep[:, c0:c1], in_=xin)

        feats = []
        for s in range(NSCALE):
            K = KS[s]
            feat = feat_pool.tile([H, T], BF16, tag=f"feat{s}", name=f"feat_{b}_{s}")
            for half in range(2):
                ps = convps_pool.tile([128, 1024], F32, tag="convps")
                for tcol in range(2):
                    t0 = half * 1024 + tcol * 512
                    glist = GROUPS[s]
                    for gi, (gg, nrows, k0, gs) in enumerate(glist):
                        n0 = t0 + PAD + k0 - K // 2
                        nc.tensor.matmul(
                            ps[:, tcol * 512 : (tcol + 1) * 512],
                            lhsT=wc_all[0:nrows, gg * H : (gg + 1) * H],
                            rhs=xrep[0:nrows, n0 : n0 + 512],
                            start=(gi == 0),
                            stop=(gi == len(glist) - 1),
                        )
                # hardware cummax: DVE scan straight out of PSUM
                # (tensor_tensor_scan is DVE-only; walrus rejects it on Pool)
                init = -1e30 if half == 0 else feat[:, 1023:1024]
                nc.vector.tensor_tensor_scan(
                    out=feat[:, half * 1024 : (half + 1) * 1024],
                    data0=ps[:, 0:1024],
                    data1=zeros[:, 0:1024],
                    initial=init,
                    op0=ALU.max,
                    op1=ALU.bypass,
                )
            feats.append(feat)

        # head: logits[t, c] for 16 t-tiles x 11 cols, one PSUM bank
        ps_head = headps_pool.tile([128, NT * NHEAD], F32, tag="headps")
        for j in range(NT):
            for s in range(NSCALE):
                nc.tensor.matmul(
                    ps_head[:, j * NHEAD : (j + 1) * NHEAD],
                    lhsT=feats[s][:, j * 128 : (j + 1) * 128],
                    rhs=whead_all[:, s * NHEAD : (s + 1) * NHEAD],
                    start=(s == 0),
                    stop=(s == NSCALE - 1),
                )

        logits = work_pool.tile([128, NT * NHEAD], F32, tag="logits")
        nc.vector.tensor_tensor(out=logits, in0=ps_head, in1=bias_bc, op=ALU.add)

        ex = work_pool.tile([128, NT * NHEAD], F32, tag="ex")
        nc.scalar.activation(ex, logits, ACTF.Exp)

        ex3 = ex.rearrange("p (j c) -> p j c", c=NHEAD)
        ssum = work_pool.tile([128, NT], F32, tag="ssum")
        nc.vector.tensor_reduce(
            out=ssum, in_=ex3[:, :, 0:NCLS], axis=AX.X, op=ALU.add
        )
        rs = work_pool.tile([128, NT], F32, tag="rs")
        nc.vector.reciprocal(out=rs, in_=ssum)

        probs_sb = work_pool.tile([128, NT * NCLS], F32, tag="probs_sb")
        rs_bc = bass.AP(
            tensor=rs.tensor,
            offset=rs.offset,
            ap=[rs.ap[0], [1, NT], [0, NCLS]],
        )
        nc.vector.tensor_tensor(
            out=probs_sb.rearrange("p (j c) -> p j c", c=NCLS),
            in0=ex3[:, :, 0:NCLS],
            in1=rs_bc,
            op=ALU.mult,
        )
        nc.sync.dma_start(
            out=probs_d[b].rearrange("(j p) c -> p j c", p=128),
            in_=probs_sb.rearrange("p (j c) -> p j c", c=NCLS),
        )

        # stage this batch's dec column (strided col 10 of each 11-group)
        lg3 = logits.rearrange("p (j c) -> p j c", c=NHEAD)
        nc.vector.tensor_copy(
            out=dcols[:, b * NT : (b + 1) * NT].rearrange("p (j o) -> p j o", o=1),
            in_=lg3[:, :, NCLS : NCLS + 1],
        )

    # ---- deltas tail: softmax over T, budget cumprod, pts ----
    # No max-subtraction: dec logits are O(+-15), exp stays in fp32 range,
    # and softmax is shift-invariant. delta = ed*rd is folded into the q
    # activation (scale AP) and the pts multiply (scalar_tensor_tensor).
    #
    # One transpose for all batches ([128, 64] -> [64, 128]); exp + per-row
    # accumulation happen in the compact [64, 128] layout (cheap on ACT),
    # then two parallel DMAs flatten values and sums into t-order rows.
    tr = trps_pool.tile([B * NT, 128], F32, tag="trps")
    nc.tensor.transpose(out=tr, in_=dcols, identity=identity)
    ed64 = singles.tile([B * NT, 128], F32)
    acc64 = singles.tile([B * NT, 1], F32)
    nc.scalar.activation(ed64, tr, ACTF.Exp, bias=0.0, scale=1.0, accum_out=acc64)

    ed = singles.tile([B, T], F32)
    nc.scalar.dma_start(out=ed.rearrange("b (j p) -> b j p", p=128), in_=ed64)
    ssum_row = singles.tile([B, NT], F32)
    nc.sync.dma_start(
        out=ssum_row.rearrange("b (j o) -> b j o", o=1), in_=acc64
    )
    ssumd = singles.tile([B, 1], F32)
    nc.vector.tensor_reduce(out=ssumd, in_=ssum_row, axis=AX.X, op=ALU.add)
    rd = singles.tile([B, 1], F32)
    nc.vector.reciprocal(out=rd, in_=ssumd)
    negrd = singles.tile([B, 1], F32)
    nc.scalar.activation(negrd, rd, ACTF.Copy, bias=0.0, scale=-1.0)

    # q = 1 - delta = 1 - ed*rd
    q = singles.tile([B, T], F32)
    nc.scalar.activation(q, ed, ACTF.Copy, bias=1.0, scale=negrd)

    sc = singles.tile([B, T - 1], F32)
    nc.vector.tensor_tensor_scan(
        out=sc,
        data0=q[:, 1:T],
        data1=zeros[0:B, 0 : T - 1],
        initial=1.0,
        op0=ALU.mult,
        op1=ALU.bypass,
    )

    pts_sb = singles.tile([B, T], F32)
    nc.vector.tensor_scalar_mul(pts_sb[:, 0:1], ed[:, 1:2], rd)
    # pts[i] = delta[i+1] * budget[i] = (ed[i+1]*rd) * sc[i-1]
    nc.vector.scalar_tensor_tensor(
        out=pts_sb[:, 1 : T - 1],
        in0=ed[:, 2:T],
        scalar=rd,
        in1=sc[:, 0 : T - 2],
        op0=ALU.mult,
        op1=ALU.mult,
    )
    nc.vector.tensor_copy(out=pts_sb[:, T - 1 : T], in_=sc[:, T - 2 : T - 1])
    nc.scalar.dma_start(out=pts_d, in_=pts_sb)


def build_nc():
    global _CACHED_NC
    if _CACHED_NC is not None:
        return _CACHED_NC
    nc = bacc.Bacc(
        "TRN2", target_bir_lowering=False, debug=False, num_devices=N_CORES
    )
    from contextlib import ExitStack

    with tile.TileContext(nc) as tc, ExitStack() as ctx:
        _emit(tc, ctx)
    nc.compile()
    _CACHED_NC = nc
    return nc


def host_prep(inputs):
    """Fold BN + biases; pack conv/head weights. Returns per-core param dict."""
    f32 = np.float32
    gamma = np.asarray(inputs["bn_gamma"], f32)
    beta = np.asarray(inputs["bn_beta"], f32)
    mean = np.asarray(inputs["bn_mean"], f32)
    var = np.asarray(inputs["bn_var"], f32)
    a = (gamma / np.sqrt(var + np.float32(1e-5))).astype(f32)  # [512]
    cshift = (beta - mean * a).astype(f32)

    ws = [np.asarray(inputs[f"w{i}"], f32) for i in range(1, 5)]
    bs = [np.asarray(inputs[f"b{i}"], f32) for i in range(1, 5)]

    cb = np.zeros(4 * H, f32)  # per-feature constant shift (conv bias + BN)
    wconv = np.zeros((13 * NREP, NGROUPS * H), f32)
    for s, (w, bias, K) in enumerate(zip(ws, bs, KS)):
        asl = a[s * H : (s + 1) * H]
        csl = cshift[s * H : (s + 1) * H]
        wp = (w * asl[:, None, None]).astype(f32)  # [H, C, K]
        cb[s * H : (s + 1) * H] = bias * asl + csl
        wt = np.ascontiguousarray(np.transpose(wp, (2, 1, 0)))  # [K, C, H]
        for gg, nrows, k0, gs in GROUPS[s]:
            wconv[:nrows, gg * H : (gg + 1) * H] = wt[k0 : k0 + gs].reshape(
                gs * C, H
            )

    logreg_w = np.asarray(inputs["logreg_w"], f32)  # [10, 512]
    logreg_b = np.asarray(inputs["logreg_b"], f32)
    dec_w = np.asarray(inputs["dec_w"], f32)  # [1, 512]

    # [128, 4*11]: chunk s at cols [s*11, (s+1)*11), bf16 for the head matmul
    whead = np.zeros((H, NSCALE * NHEAD), f32)
    for s in range(NSCALE):
        whead[:, s * NHEAD : s * NHEAD + NCLS] = logreg_w.T[s * H : (s + 1) * H]
        whead[:, s * NHEAD + NCLS] = dec_w[0, s * H : (s + 1) * H]
    whead = whead.astype(mybir.dt.np(BF16))

    hb = np.zeros(NHEAD, f32)
    hb[:NCLS] = logreg_b + logreg_w @ cb  # dec-col bias is softmax-invariant
    biasbc = np.broadcast_to(np.tile(hb, NT), (128, NT * NHEAD))
    biasbc = np.ascontiguousarray(biasbc, dtype=f32)

    return {"wconv": wconv, "whead": whead, "biasbc": biasbc}


def make_in_maps(inputs):
    params = host_prep(inputs)
    x = np.asarray(inputs["x"], np.float32)
    xpad = np.zeros((B_FULL, C, XPAD_W), np.float32)
    xpad[:, :, PAD : PAD + T] = x
    in_maps = []
    for i in range(N_CORES):
        m = {"x": np.ascontiguousarray(xpad[i * B : (i + 1) * B])}
        m.update(params)
        in_maps.append(m)
    return in_maps


def kernel(**inputs):
    global LAST_EXEC_NS
    nc = build_nc()
    in_maps = make_in_maps(inputs)
    res = run_bass_kernel_spmd(nc, in_maps, list(range(N_CORES)))
    LAST_EXEC_NS = res.exec_time_ns
    probs = np.concatenate([res.results[i]["probs"] for i in range(N_CORES)], 0)
    pts = np.concatenate([res.results[i]["pts"] for i in range(N_CORES)], 0)
    return probs, pts


def bench(inputs, iters=64, warmup=8):
    """Amortized per-call wall time of the compiled NEFF across 8 cores.

    No NTFF profiling is available through the axon tunnel in this
    container, so this times back-to-back PJRT executions with inputs
    resident on device (no donation, outputs written fully by the
    kernel) and reports the steady-state per-call time.
    """
    import time

    import jax
    from jax.sharding import Mesh, PartitionSpec
    from jax.experimental.shard_map import shard_map

    from concourse import bass2jax
    from concourse import mybir as mb

    nc = build_nc()
    in_maps = make_in_maps(inputs)
    bass2jax.install_neuronx_cc_hook()

    partition_name = (
        nc.partition_id_tensor.name if nc.partition_id_tensor else None
    )
    in_names, out_names, out_avals, zero_outs = [], [], [], []
    for alloc in nc.m.functions[0].allocations:
        if not isinstance(alloc, mb.MemoryLocationSet):
            continue
        name = alloc.memorylocations[0].name
        if alloc.kind == "ExternalInput":
            if name != partition_name:
                in_names.append(name)
        elif alloc.kind == "ExternalOutput":
            out_names.append(name)
            out_avals.append(
                jax.core.ShapedArray(alloc.tensor_shape, mb.dt.np(alloc.dtype))
            )
            zero_outs.append(np.zeros(alloc.tensor_shape, mb.dt.np(alloc.dtype)))
    n_params = len(in_names)
    all_names = in_names + out_names
    if partition_name is not None:
        all_names = all_names + [partition_name]

    def _body(*args):
        operands = list(args)
        if partition_name is not None:
            operands.append(bass2jax.partition_id_tensor())
        outs = bass2jax._bass_exec_p.bind(
            *operands,
            out_avals=tuple(out_avals),
            in_names=tuple(all_names),
            out_names=tuple(out_names),
            lowering_input_output_aliases=(),
            sim_require_finite=True,
            sim_require_nnan=True,
            nc=nc,
        )
        return tuple(outs)

    devices = jax.devices()[:N_CORES]
    mesh = Mesh(np.asarray(devices), ("core",))
    nin = n_params + len(out_names)
    sharded = jax.jit(
        shard_map(
            _body,
            mesh=mesh,
            in_specs=(PartitionSpec("core"),) * nin,
            out_specs=(PartitionSpec("core"),) * len(out_names),
            check_rep=False,
        ),
        keep_unused=True,
    )
    # shard along axis0: per-core shard must equal the declared per-core shape
    concat_in = [
        np.concatenate([np.asarray(in_maps[c][n]) for c in range(N_CORES)], 0)
        for n in in_names
    ]
    concat_zeros = [
        np.zeros((N_CORES * z.shape[0], *z.shape[1:]), z.dtype) for z in zero_outs
    ]
    from jax.sharding import NamedSharding

    sh = NamedSharding(mesh, PartitionSpec("core"))
    dev_in = [jax.device_put(a, sh) for a in concat_in + concat_zeros]

    for _ in range(warmup):
        out = sharded(*dev_in)
    jax.block_until_ready(out)
    t0 = time.perf_counter()
    for _ in range(iters):
        out = sharded(*dev_in)
    jax.block_until_ready(out)
    t1 = time.perf_counter()
    return (t1 - t0) / iters * 1e9
